# revision 24
# baseline (speedup 1.0000x reference)
"""CondenseMSA segment-reduce kernel for Trainium2 (8 NeuronCores).

The reference computes:
  emb = concat(E_aa[X], features)           [B,T,A,64]
  BatchNorm (train-mode global stats) -> lin1 -> mean over A -> [B,T,64]
  per-batch scatter-add by `focuses` into [max_seq_len, 64], divide by counts.

Everything before the divide is LINEAR in the one-hot encodings of X and
focuses, so the device only needs cheap streaming sums:

  per shard (b, half-of-T):  t-chunks of 128 on partitions
    n[t, aa]  = #{a : X[t,a]==aa}                 (compare + reduce)
    fs[t, f]  = sum_a features[t,a,f]             (reduce)
    ohF[t, p] = (focuses[t]==p)                   (iota compare)
    PSUM[37,400] += [n | fs].T @ ohF              (PE matmul, accumulated)
    PSUM[1,256]  += ones.T @ features^2           (BN variance partial)

Host then applies the tiny affine fix-up (BN stats from histogram/momemts,
lin1 folding, count division) on [4,400,64] — a few MFLOPs.

Sharding: 8 cores = (4 batches) x (2 halves of T). No cross-core comms;
partial sums combined on host.
"""

import os
import numpy as np
from contextlib import ExitStack

import concourse.bass as bass
import concourse.bacc as bacc
import concourse.tile as tile
from concourse import mybir
from concourse import bass_utils
from concourse._compat import with_exitstack

# ---- problem constants (hardcoded; kernel.py must be self-contained) ----
B = 4
T = 12800
A = 16
F = 16
NAA = 21
H = 64
AF = A * F          # 256
NPOS = 400          # focuses < 400
MW = NAA + F        # 37
MAX_SEQ_LEN = 500
BN_EPS = 1e-5
P = 128
N_CORES = 8
TSH = T // 2        # 6400 t per shard
NCHUNK = TSH // P   # 50

f32 = mybir.dt.float32
f32r = mybir.dt.float32r
bf16 = mybir.dt.bfloat16
i16 = mybir.dt.int16

# toggles for perf experiments
AAEQ_ENGINE = os.environ.get("K_AAEQ_ENGINE", "vector")  # gpsimd | vector | alternate
OHF_ENGINE = os.environ.get("K_OHF_ENGINE", "gpsimd")    # gpsimd | vector
MM_DTYPE = os.environ.get("K_MM_DTYPE", "f32r")          # f32r | f32
WORK_BUFS = int(os.environ.get("K_WORK_BUFS", "12"))

TRACE = False
LAST_RESULTS = None


@with_exitstack
def _kernel_body(ctx: ExitStack, tc: tile.TileContext, ins, outs):
    """Device kernel, per 2-chunk pair (128 t on partitions per chunk):
      DMA:  featc2 [128,2,256] f32r (one 256KB load per pair)
      ACT:  sqt2 = featc2^2 (one Square per pair)
      DVE:  oh2[t,k,aa,a] = (X==aa) compare (bf16), reduce_sum over a -> Mn2
      Pool: ohF[t,p] = (focus[t]==p) f32r [128,400] (per chunk)
      PE:   psum_n  [21,400]  += Mn.T @ ohF
            psum_f1 [128,400] += feat[:, :128].T @ ohF   (raw feature scatter;
            psum_f2 [128,400] += feat[:, 128:].T @ ohF    host reduces the a-dim)
            psum_sq [1,512]   += ones.T @ sqt2           (BN variance partial)

    All matmul operands are float32r (full-rate 1 cyc/row on PE for moving
    dims >= 256; plain fp32 streams at 1/4 rate). Engine balance per shard
    (cost model): PE ~32us, DVE ~29us, DMA ~29us, ACT ~18us, Pool ~17us;
    PE runs gap-free. Host applies the tiny affine BN/lin1 fix-up.
    """
    nc = tc.nc
    feat, aux16, auxf, onesw = ins
    agg_n_out, f1_out, f2_out, sq_out = outs

    singles = ctx.enter_context(tc.tile_pool(name="singles", bufs=1))
    work = ctx.enter_context(tc.tile_pool(name="work", bufs=WORK_BUFS))
    psum = ctx.enter_context(tc.tile_pool(name="psum", bufs=1, space="PSUM"))

    # one-time loads (each aux is ONE DMA so consumers carry at most one wait)
    aux16_sb = singles.tile([P, NCHUNK * A + NAA * A], i16)
    nc.sync.dma_start(out=aux16_sb, in_=aux16)
    auxf_sb = singles.tile([P, NCHUNK + NPOS], f32)
    nc.sync.dma_start(out=auxf_sb, in_=auxf)
    xprep_sb = aux16_sb[:, 0:NCHUNK * A]
    pattern_sb = aux16_sb[:, NCHUNK * A:].rearrange("p (aa a) -> p aa a", a=A)
    focus_sb = auxf_sb[:, 0:NCHUNK]
    iota_sb = auxf_sb[:, NCHUNK:]
    mmdt = f32r if MM_DTYPE == "f32r" else f32
    feat = feat.bitcast(mmdt)

    ones_sb = singles.tile([P, 1], mmdt)
    nc.sync.dma_start(out=ones_sb, in_=onesw.bitcast(mmdt))

    psum_n = psum.tile([NAA, NPOS], f32)
    psum_f1 = psum.tile([P, NPOS], f32)
    psum_f2 = psum.tile([P, NPOS], f32)
    psum_sq = psum.tile([1, 2 * AF], f32)

    feat_t2 = feat.rearrange("(cc k p) d -> cc p k d", k=2, p=P)  # [25,128,2,256]

    for cc in range(NCHUNK // 2):
        featc2 = work.tile([P, 2, AF], mmdt)
        nc.sync.dma_start(out=featc2, in_=feat_t2[cc])

        # squares for BN variance (ScalarE), both chunks in one op
        sqt2 = work.tile([P, 2, AF], mmdt)
        nc.scalar.activation(
            out=sqt2[:].rearrange("p k d -> p (k d)"),
            in_=featc2[:].rearrange("p k d -> p (k d)"),
            func=mybir.ActivationFunctionType.Square,
        )

        # AA one-hot both chunks: oh2[t, k, aa, a] = (X[t, k, a] == aa)
        oh2 = work.tile([P, 2, NAA, A], bf16)
        xs2 = xprep_sb[:, 2 * cc * A:(2 * cc + 2) * A].rearrange(
            "p (k a) -> p k a", k=2
        )
        xb2 = xs2.unsqueeze(2).broadcast_to([P, 2, NAA, A])
        pat2 = pattern_sb.unsqueeze(1).broadcast_to([P, 2, NAA, A])
        if AAEQ_ENGINE == "alternate":
            eng = nc.gpsimd if (cc % 3 == 0) else nc.vector
        else:
            eng = nc.gpsimd if AAEQ_ENGINE == "gpsimd" else nc.vector
        eng.tensor_tensor(
            out=oh2[:], in0=xb2, in1=pat2, op=mybir.AluOpType.is_equal
        )
        Mn2 = work.tile([P, 2, NAA], mmdt)
        with nc.allow_low_precision(reason="values are small exact integers"):
            nc.vector.reduce_sum(out=Mn2[:], in_=oh2[:], axis=mybir.AxisListType.X)

        for k in range(2):
            c = 2 * cc + k
            # focus one-hot (GpSimd, frees the DVE)
            ohF = work.tile([P, NPOS], mmdt)
            ohf_eng = nc.gpsimd if OHF_ENGINE == "gpsimd" else nc.vector
            ohf_eng.tensor_scalar(
                out=ohF[:],
                in0=iota_sb[:],
                scalar1=focus_sb[:, c:c + 1],
                scalar2=None,
                op0=mybir.AluOpType.is_equal,
            )
            st = dict(start=(c == 0), stop=(c == NCHUNK - 1))
            nc.tensor.matmul(out=psum_n[:], lhsT=Mn2[:, k, :], rhs=ohF[:], **st)
            nc.tensor.matmul(out=psum_f1[:], lhsT=featc2[:, k, 0:P], rhs=ohF[:], **st)
            nc.tensor.matmul(out=psum_f2[:], lhsT=featc2[:, k, P:AF], rhs=ohF[:], **st)

        stcc = dict(start=(cc == 0), stop=(cc == NCHUNK // 2 - 1))
        nc.tensor.matmul(
            out=psum_sq[:],
            lhsT=ones_sb[:],
            rhs=sqt2[:].rearrange("p k d -> p (k d)"),
            **stcc,
        )

    def _copy_act(sb, psrc):
        nc.scalar.activation(
            out=sb[:], in_=psrc[:], func=mybir.ActivationFunctionType.Copy
        )

    def _copy_dve(sb, psrc):
        nc.vector.tensor_copy(out=sb[:], in_=psrc[:])

    for psrc, dst, cp, dma in (
        (psum_n, agg_n_out, _copy_act, nc.sync),
        (psum_f1, f1_out, _copy_dve, nc.sync),
        (psum_f2, f2_out, _copy_dve, nc.sync),
        (psum_sq, sq_out, _copy_act, nc.sync),
    ):
        sb = singles.tile(list(psrc.shape), f32, tag=f"out_{dst.name}")
        cp(sb, psrc)
        dma.dma_start(out=dst, in_=sb[:])


_BUILD_CACHE = None


def _build():
    global _BUILD_CACHE
    if _BUILD_CACHE is not None:
        return _BUILD_CACHE
    nc = bacc.Bacc("TRN2", target_bir_lowering=False, debug=False)
    feat = nc.dram_tensor("feat", [TSH, AF], f32, kind="ExternalInput").ap()
    aux16 = nc.dram_tensor("aux16", [P, NCHUNK * A + NAA * A], i16, kind="ExternalInput").ap()
    auxf = nc.dram_tensor("auxf", [P, NCHUNK + NPOS], f32, kind="ExternalInput").ap()
    onesw = nc.dram_tensor("onesw", [P, 1], f32, kind="ExternalInput").ap()
    agg_n_out = nc.dram_tensor("agg_n_out", [NAA, NPOS], f32, kind="ExternalOutput").ap()
    f1_out = nc.dram_tensor("f1_out", [P, NPOS], f32, kind="ExternalOutput").ap()
    f2_out = nc.dram_tensor("f2_out", [P, NPOS], f32, kind="ExternalOutput").ap()
    sq_out = nc.dram_tensor("sq_out", [1, 2 * AF], f32, kind="ExternalOutput").ap()
    with tile.TileContext(nc) as tc:
        _kernel_body(tc, (feat, aux16, auxf, onesw), (agg_n_out, f1_out, f2_out, sq_out))
    nc.compile()
    _BUILD_CACHE = nc
    return nc


def _host_prep(X, features, focuses):
    """Build per-core input maps."""
    X16 = np.ascontiguousarray(X.astype(np.int16))
    foc32 = np.ascontiguousarray(focuses.astype(np.float32))
    pattern = np.tile(
        np.repeat(np.arange(NAA, dtype=np.int16), A)[None, :], (P, 1)
    )
    iota = np.tile(np.arange(NPOS, dtype=np.float32)[None, :], (P, 1))
    in_maps = []
    for k in range(N_CORES):
        b, half = k // 2, k % 2
        t0, t1 = half * TSH, (half + 1) * TSH
        xs = X16[b, t0:t1]  # [6400, 16]
        xprep = xs.reshape(NCHUNK, P, A).transpose(1, 0, 2).reshape(P, NCHUNK * A)
        focusp = foc32[b, t0:t1].reshape(NCHUNK, P).T
        aux16 = np.ascontiguousarray(np.concatenate([xprep, pattern], axis=1))
        auxf = np.ascontiguousarray(np.concatenate([focusp, iota], axis=1))
        featk = np.ascontiguousarray(
            features[b, t0:t1].reshape(TSH, AF).astype(np.float32)
        )
        in_maps.append(dict(feat=featk, aux16=aux16, auxf=auxf,
                            onesw=np.ones((P, 1), np.float32)))
    return in_maps


def _host_combine(results, inputs):
    """Affine fix-up + count division on host (tiny)."""
    E_aa = np.asarray(inputs["E_aa"], dtype=np.float32)
    bn_gamma = np.asarray(inputs["bn_gamma"], dtype=np.float32)
    bn_beta = np.asarray(inputs["bn_beta"], dtype=np.float32)
    lin1_W = np.asarray(inputs["lin1_W"], dtype=np.float32)
    lin1_b = np.asarray(inputs["lin1_b"], dtype=np.float32)
    seq_lens = np.asarray(inputs["seq_lens"])
    max_seq_len = int(inputs["max_seq_len"])
    N = B * T * A

    agg_n8 = np.stack([r["agg_n_out"] for r in results])   # [8, 21, 400]
    fraw8 = np.stack(
        [np.concatenate([r["f1_out"], r["f2_out"]], axis=0) for r in results]
    )                                                      # [8, 256, 400]
    sqsum = np.stack([r["sq_out"][0].reshape(2, AF).sum(0) for r in results])  # [8,256]

    agg_n = agg_n8.reshape(B, 2, NAA, NPOS).sum(1)         # [B, 21, 400]
    # raw feature scatter is per (a,f): reduce the alignment dim on host
    agg_f = fraw8.reshape(B, 2, A, F, NPOS).sum((1, 2))    # [B, 16, 400]
    count = agg_n.sum(1) / A                               # [B, 400]
    hist = agg_n.sum((0, 2))
    featsum = agg_f.sum((0, 2))
    sq_f = sqsum.sum(0).reshape(A, F).sum(0)

    mean_e = hist @ E_aa / N
    var_e = hist @ (E_aa ** 2) / N - mean_e ** 2
    mean_f = featsum / N
    var_f = sq_f / N - mean_f ** 2
    mean = np.concatenate([mean_e, mean_f])
    var = np.concatenate([var_e, var_f])

    s = bn_gamma / np.sqrt(var + BN_EPS)
    c = bn_beta - mean * s
    W1 = s[:, None] * lin1_W.T
    b1 = c @ lin1_W.T + lin1_b

    emb_part = np.einsum("bap,ac->bpc", agg_n, E_aa)
    Mavg = np.concatenate([emb_part, agg_f.transpose(0, 2, 1)], axis=2) / A
    aggregate = Mavg @ W1 + count[..., None] * b1[None, None, :]

    out = np.zeros((B, max_seq_len, H), np.float32)
    out[:, :NPOS] = aggregate
    cnt = np.zeros((B, max_seq_len), np.float32)
    cnt[:, :NPOS] = count
    pos = np.arange(max_seq_len)[None, :]
    cnt = np.where(pos >= seq_lens[:, None], 1.0, cnt)
    with np.errstate(divide="ignore", invalid="ignore"):
        out = out / cnt[..., None]
    return out.astype(np.float32)


def kernel(**inputs) -> np.ndarray:
    global LAST_RESULTS
    X = np.asarray(inputs["X"])
    features = np.asarray(inputs["features"])
    focuses = np.asarray(inputs["focuses"])

    nc = _build()
    in_maps = _host_prep(X, features, focuses)
    res = bass_utils.run_bass_kernel_spmd(
        nc, in_maps, core_ids=list(range(N_CORES)), trace=TRACE
    )
    LAST_RESULTS = res
    return _host_combine(res.results, inputs)


# revision 30
# speedup vs baseline: 1.0526x; 1.0526x over previous
"""CondenseMSA segment-reduce kernel for Trainium2 (8 NeuronCores).

The reference computes:
  emb = concat(E_aa[X], features)           [B,T,A,64]
  BatchNorm (train-mode global stats) -> lin1 -> mean over A -> [B,T,64]
  per-batch scatter-add by `focuses` into [max_seq_len, 64], divide by counts.

Everything before the divide is LINEAR in the one-hot encodings of X and
focuses, so the device only needs cheap streaming sums:

  per shard (b, half-of-T):  t-chunks of 128 on partitions
    n[t, aa]  = #{a : X[t,a]==aa}                 (compare + reduce)
    fs[t, f]  = sum_a features[t,a,f]             (reduce)
    ohF[t, p] = (focuses[t]==p)                   (iota compare)
    PSUM[37,400] += [n | fs].T @ ohF              (PE matmul, accumulated)
    PSUM[1,256]  += ones.T @ features^2           (BN variance partial)

Host then applies the tiny affine fix-up (BN stats from histogram/momemts,
lin1 folding, count division) on [4,400,64] — a few MFLOPs.

Sharding: 8 cores = (4 batches) x (2 halves of T). No cross-core comms;
partial sums combined on host.
"""

import os
import numpy as np
from contextlib import ExitStack

import concourse.bass as bass
import concourse.bacc as bacc
import concourse.tile as tile
from concourse import mybir
from concourse import bass_utils
from concourse._compat import with_exitstack

# ---- problem constants (hardcoded; kernel.py must be self-contained) ----
B = 4
T = 12800
A = 16
F = 16
NAA = 21
H = 64
AF = A * F          # 256
NPOS = 400          # focuses < 400
MW = NAA + F        # 37
MAX_SEQ_LEN = 500
BN_EPS = 1e-5
P = 128
N_CORES = 8
TSH = T // 2        # 6400 t per shard
NCHUNK = TSH // P   # 50

f32 = mybir.dt.float32
f32r = mybir.dt.float32r
bf16 = mybir.dt.bfloat16
i16 = mybir.dt.int16

# toggles for perf experiments
AAEQ_ENGINE = os.environ.get("K_AAEQ_ENGINE", "vector")  # gpsimd | vector | alternate
OHF_ENGINE = os.environ.get("K_OHF_ENGINE", "gpsimd")    # gpsimd | vector
MM_DTYPE = os.environ.get("K_MM_DTYPE", "f32r")          # f32r | f32
WORK_BUFS = int(os.environ.get("K_WORK_BUFS", "12"))

TRACE = False
LAST_RESULTS = None


@with_exitstack
def _kernel_body(ctx: ExitStack, tc: tile.TileContext, ins, outs):
    """Device kernel, per 2-chunk pair (128 t on partitions per chunk):
      DMA:  featc2 [128,2,256] f32r (one 256KB load per pair)
      ACT:  sqt2 = featc2^2 (one Square per pair)
      DVE:  oh2[t,k,aa,a] = (X==aa) compare (bf16), reduce_sum over a -> Mn2
      Pool: ohF[t,p] = (focus[t]==p) f32r [128,400] (per chunk)
      PE:   psum_n  [21,400]  += Mn.T @ ohF
            psum_f1 [128,400] += feat[:, :128].T @ ohF   (raw feature scatter;
            psum_f2 [128,400] += feat[:, 128:].T @ ohF    host reduces the a-dim)
            psum_sq [1,512]   += ones.T @ sqt2           (BN variance partial)

    All matmul operands are float32r (full-rate 1 cyc/row on PE for moving
    dims >= 256; plain fp32 streams at 1/4 rate). Engine balance per shard
    (cost model): PE ~32us, DVE ~29us, DMA ~29us, ACT ~18us, Pool ~17us;
    PE runs gap-free. Host applies the tiny affine BN/lin1 fix-up.
    """
    nc = tc.nc
    feat, aux16, auxf, onesw = ins
    agg_n_out, f1_out, f2_out, sq_out = outs

    singles = ctx.enter_context(tc.tile_pool(name="singles", bufs=1))
    work = ctx.enter_context(tc.tile_pool(name="work", bufs=WORK_BUFS))
    psum = ctx.enter_context(tc.tile_pool(name="psum", bufs=1, space="PSUM"))

    # one-time loads; pair-0 features hoisted first (queue order gates the
    # pipeline start -- measured best of the issue-order permutations)
    mmdt0 = f32r if MM_DTYPE == "f32r" else f32
    feat0 = feat.bitcast(mmdt0).rearrange("(cc k p) d -> cc p k d", k=2, p=P)
    featc2_first = work.tile([P, 2, AF], mmdt0, tag="featc2")
    nc.sync.dma_start(out=featc2_first, in_=feat0[0])
    aux16_sb = singles.tile([P, NCHUNK * A + NAA * A], i16)
    nc.sync.dma_start(out=aux16_sb, in_=aux16)
    auxf_sb = singles.tile([P, NCHUNK + NPOS], f32)
    nc.sync.dma_start(out=auxf_sb, in_=auxf)
    xprep_sb = aux16_sb[:, 0:NCHUNK * A]
    pattern_sb = aux16_sb[:, NCHUNK * A:].rearrange("p (aa a) -> p aa a", a=A)
    focus_sb = auxf_sb[:, 0:NCHUNK]
    iota_sb = auxf_sb[:, NCHUNK:]
    mmdt = f32r if MM_DTYPE == "f32r" else f32
    feat = feat.bitcast(mmdt)

    ones_sb = singles.tile([P, 1], mmdt)
    nc.sync.dma_start(out=ones_sb, in_=onesw.bitcast(mmdt))

    psum_n = psum.tile([NAA, NPOS], f32)
    psum_f1 = psum.tile([P, NPOS], f32)
    psum_f2 = psum.tile([P, NPOS], f32)
    psum_sq = psum.tile([1, 2 * AF], f32)

    feat_t2 = feat.rearrange("(cc k p) d -> cc p k d", k=2, p=P)  # [25,128,2,256]

    for cc in range(NCHUNK // 2):
        if cc == 0:
            featc2 = featc2_first
        else:
            featc2 = work.tile([P, 2, AF], mmdt, tag="featc2")
            nc.sync.dma_start(out=featc2, in_=feat_t2[cc])

        # squares for BN variance (ScalarE), both chunks in one op
        sqt2 = work.tile([P, 2, AF], mmdt)
        nc.scalar.activation(
            out=sqt2[:].rearrange("p k d -> p (k d)"),
            in_=featc2[:].rearrange("p k d -> p (k d)"),
            func=mybir.ActivationFunctionType.Square,
        )

        # AA one-hot both chunks: oh2[t, k, aa, a] = (X[t, k, a] == aa)
        oh2 = work.tile([P, 2, NAA, A], bf16)
        xs2 = xprep_sb[:, 2 * cc * A:(2 * cc + 2) * A].rearrange(
            "p (k a) -> p k a", k=2
        )
        xb2 = xs2.unsqueeze(2).broadcast_to([P, 2, NAA, A])
        pat2 = pattern_sb.unsqueeze(1).broadcast_to([P, 2, NAA, A])
        if AAEQ_ENGINE == "alternate":
            eng = nc.gpsimd if (cc % 3 == 0) else nc.vector
        else:
            eng = nc.gpsimd if AAEQ_ENGINE == "gpsimd" else nc.vector
        eng.tensor_tensor(
            out=oh2[:], in0=xb2, in1=pat2, op=mybir.AluOpType.is_equal
        )
        Mn2 = work.tile([P, 2, NAA], mmdt)
        with nc.allow_low_precision(reason="values are small exact integers"):
            nc.vector.reduce_sum(out=Mn2[:], in_=oh2[:], axis=mybir.AxisListType.X)

        for k in range(2):
            c = 2 * cc + k
            # focus one-hot (GpSimd, frees the DVE)
            ohF = work.tile([P, NPOS], mmdt)
            ohf_eng = nc.gpsimd if OHF_ENGINE == "gpsimd" else nc.vector
            ohf_eng.tensor_scalar(
                out=ohF[:],
                in0=iota_sb[:],
                scalar1=focus_sb[:, c:c + 1],
                scalar2=None,
                op0=mybir.AluOpType.is_equal,
            )
            st = dict(start=(c == 0), stop=(c == NCHUNK - 1))
            nc.tensor.matmul(out=psum_n[:], lhsT=Mn2[:, k, :], rhs=ohF[:], **st)
            nc.tensor.matmul(out=psum_f1[:], lhsT=featc2[:, k, 0:P], rhs=ohF[:], **st)
            nc.tensor.matmul(out=psum_f2[:], lhsT=featc2[:, k, P:AF], rhs=ohF[:], **st)

        stcc = dict(start=(cc == 0), stop=(cc == NCHUNK // 2 - 1))
        nc.tensor.matmul(
            out=psum_sq[:],
            lhsT=ones_sb[:],
            rhs=sqt2[:].rearrange("p k d -> p (k d)"),
            **stcc,
        )

    def _copy_act(sb, psrc):
        nc.scalar.activation(
            out=sb, in_=psrc, func=mybir.ActivationFunctionType.Copy
        )

    def _copy_dve(sb, psrc):
        nc.vector.tensor_copy(out=sb, in_=psrc)

    for psrc, dst, cp in (
        (psum_n, agg_n_out, _copy_act),
        (psum_f1, f1_out, _copy_dve),
        (psum_f2, f2_out, _copy_dve),
        (psum_sq, sq_out, _copy_act),
    ):
        sb = singles.tile(list(psrc.shape), f32, tag=f"out_{dst.name}")
        cp(sb[:], psrc[:])
        nc.sync.dma_start(out=dst, in_=sb[:])


_BUILD_CACHE = None


def _build():
    global _BUILD_CACHE
    if _BUILD_CACHE is not None:
        return _BUILD_CACHE
    nc = bacc.Bacc("TRN2", target_bir_lowering=False, debug=False)
    feat = nc.dram_tensor("feat", [TSH, AF], f32, kind="ExternalInput").ap()
    aux16 = nc.dram_tensor("aux16", [P, NCHUNK * A + NAA * A], i16, kind="ExternalInput").ap()
    auxf = nc.dram_tensor("auxf", [P, NCHUNK + NPOS], f32, kind="ExternalInput").ap()
    onesw = nc.dram_tensor("onesw", [P, 1], f32, kind="ExternalInput").ap()
    agg_n_out = nc.dram_tensor("agg_n_out", [NAA, NPOS], f32, kind="ExternalOutput").ap()
    f1_out = nc.dram_tensor("f1_out", [P, NPOS], f32, kind="ExternalOutput").ap()
    f2_out = nc.dram_tensor("f2_out", [P, NPOS], f32, kind="ExternalOutput").ap()
    sq_out = nc.dram_tensor("sq_out", [1, 2 * AF], f32, kind="ExternalOutput").ap()
    with tile.TileContext(nc) as tc:
        _kernel_body(tc, (feat, aux16, auxf, onesw), (agg_n_out, f1_out, f2_out, sq_out))
    nc.compile()
    _BUILD_CACHE = nc
    return nc


def _host_prep(X, features, focuses):
    """Build per-core input maps."""
    X16 = np.ascontiguousarray(X.astype(np.int16))
    foc32 = np.ascontiguousarray(focuses.astype(np.float32))
    pattern = np.tile(
        np.repeat(np.arange(NAA, dtype=np.int16), A)[None, :], (P, 1)
    )
    iota = np.tile(np.arange(NPOS, dtype=np.float32)[None, :], (P, 1))
    in_maps = []
    for k in range(N_CORES):
        b, half = k // 2, k % 2
        t0, t1 = half * TSH, (half + 1) * TSH
        xs = X16[b, t0:t1]  # [6400, 16]
        xprep = xs.reshape(NCHUNK, P, A).transpose(1, 0, 2).reshape(P, NCHUNK * A)
        focusp = foc32[b, t0:t1].reshape(NCHUNK, P).T
        aux16 = np.ascontiguousarray(np.concatenate([xprep, pattern], axis=1))
        auxf = np.ascontiguousarray(np.concatenate([focusp, iota], axis=1))
        featk = np.ascontiguousarray(
            features[b, t0:t1].reshape(TSH, AF).astype(np.float32)
        )
        in_maps.append(dict(feat=featk, aux16=aux16, auxf=auxf,
                            onesw=np.ones((P, 1), np.float32)))
    return in_maps


def _host_combine(results, inputs):
    """Affine fix-up + count division on host (tiny)."""
    E_aa = np.asarray(inputs["E_aa"], dtype=np.float32)
    bn_gamma = np.asarray(inputs["bn_gamma"], dtype=np.float32)
    bn_beta = np.asarray(inputs["bn_beta"], dtype=np.float32)
    lin1_W = np.asarray(inputs["lin1_W"], dtype=np.float32)
    lin1_b = np.asarray(inputs["lin1_b"], dtype=np.float32)
    seq_lens = np.asarray(inputs["seq_lens"])
    max_seq_len = int(inputs["max_seq_len"])
    N = B * T * A

    agg_n8 = np.stack([r["agg_n_out"] for r in results])   # [8, 21, 400]
    fraw8 = np.stack(
        [np.concatenate([r["f1_out"], r["f2_out"]], axis=0) for r in results]
    )                                                      # [8, 256, 400]
    sqsum = np.stack([r["sq_out"][0].reshape(2, AF).sum(0) for r in results])  # [8,256]

    agg_n = agg_n8.reshape(B, 2, NAA, NPOS).sum(1)         # [B, 21, 400]
    # raw feature scatter is per (a,f): reduce the alignment dim on host
    agg_f = fraw8.reshape(B, 2, A, F, NPOS).sum((1, 2))    # [B, 16, 400]
    count = agg_n.sum(1) / A                               # [B, 400]
    hist = agg_n.sum((0, 2))
    featsum = agg_f.sum((0, 2))
    sq_f = sqsum.sum(0).reshape(A, F).sum(0)

    mean_e = hist @ E_aa / N
    var_e = hist @ (E_aa ** 2) / N - mean_e ** 2
    mean_f = featsum / N
    var_f = sq_f / N - mean_f ** 2
    mean = np.concatenate([mean_e, mean_f])
    var = np.concatenate([var_e, var_f])

    s = bn_gamma / np.sqrt(var + BN_EPS)
    c = bn_beta - mean * s
    W1 = s[:, None] * lin1_W.T
    b1 = c @ lin1_W.T + lin1_b

    emb_part = np.einsum("bap,ac->bpc", agg_n, E_aa)
    Mavg = np.concatenate([emb_part, agg_f.transpose(0, 2, 1)], axis=2) / A
    aggregate = Mavg @ W1 + count[..., None] * b1[None, None, :]

    out = np.zeros((B, max_seq_len, H), np.float32)
    out[:, :NPOS] = aggregate
    cnt = np.zeros((B, max_seq_len), np.float32)
    cnt[:, :NPOS] = count
    pos = np.arange(max_seq_len)[None, :]
    cnt = np.where(pos >= seq_lens[:, None], 1.0, cnt)
    with np.errstate(divide="ignore", invalid="ignore"):
        out = out / cnt[..., None]
    return out.astype(np.float32)


def kernel(**inputs) -> np.ndarray:
    global LAST_RESULTS
    X = np.asarray(inputs["X"])
    features = np.asarray(inputs["features"])
    focuses = np.asarray(inputs["focuses"])

    nc = _build()
    in_maps = _host_prep(X, features, focuses)
    res = bass_utils.run_bass_kernel_spmd(
        nc, in_maps, core_ids=list(range(N_CORES)), trace=TRACE
    )
    LAST_RESULTS = res
    return _host_combine(res.results, inputs)
